# revision 1
# baseline (speedup 1.0000x reference)
"""DockingScorePredictor Trainium2 kernel.

Data-parallel over complexes: 8 cores, one complex (512 protein x 64 ligand
atoms) per core.  Per core the pair-MLP runs as 64 tiles of 512 pairs (one
ligand atom per tile, all 512 protein atoms), activations feature-major
[H=128 partitions, pairs on free dim].

Per tile (l = ligand atom):
  z1 = W1a.T @ hpT                (precomputed once; identity-matmul add)
     + W1c.T @ rbT                (K=32 matmul on 4-tile stacked radial basis)
     + (W1b.T @ hlT + b1)[:, l]   (free via relu bias port)
  a1 = relu(z1 + bias_l)
  a2 = relu(W2.T a1 + b2)
  z3 = W3.T a2 - 1e9*notmask      (K=1 inject matmul kills masked pairs)
  relu3 + pair-sum                (accum_out port)
Relu layers alternate ACT/DVE by tile parity; radial-basis affine+square on
Pool, exp on ACT.  Host precomputes exact fp32 pair distances, the 0/1
notmask, and 1/count (cheap O(pairs) coordinate prep; the 3 GFLOP MLP stays
on device).  MLP matmuls are float32r (1 col/cycle at N=512).  Emission is
software-pipelined ~5 stages deep so PE never waits on relus.
"""
import numpy as np
from contextlib import ExitStack

import concourse.bass as bass
import concourse.bacc as bacc
import concourse.tile as tile
from concourse import mybir
from concourse import bass_utils

F32 = mybir.dt.float32
F32R = mybir.dt.float32r
AF = mybir.ActivationFunctionType
ALU = mybir.AluOpType

B, P, L = 8, 512, 64
H, RB = 128, 32
CUTOFF = 8.0
N_CORES = 8
NPAIR = P * L
TILES = L
GROUPS = TILES // 4
WIDTH = 0.5 * CUTOFF / RB + 1e-8

_CACHE = {}


def _build_nc():
    nc = bacc.Bacc("TRN2", target_bir_lowering=False, debug=False,
                   num_devices=N_CORES)
    d = {}

    def inp(name, shape, dt):
        d[name] = nc.dram_tensor(name, shape, dt, kind="ExternalInput").ap()

    inp("hpT", [H, P], F32R)
    inp("hlT", [H, L], F32R)
    inp("dbpre", [H, 512 * GROUPS], F32)
    inp("nmpre", [H, 512 * GROUPS], F32R)  # rows 32s: notmask, rows 32s+1: 1.0, rest 0
    inp("W1a", [H, H], F32R)
    inp("W1b", [H, H], F32R)
    inp("W1csx", [H, 4 * H], F32R)    # 4 variants: W1c at rows 32s, zeros else
    inp("W2", [H, H], F32R)
    inp("W3", [H, H], F32R)
    inp("Wr1", [H, H], F32)
    inp("Wr2", [H, 1], F32)
    inp("negb3x", [H, 4 * H], F32R)   # 4 variants: row 32s=-1e9, row 32s+1=b3
    inp("onesr", [1, 512], F32R)
    inp("id128", [H, H], F32R)
    inp("b1", [H, 1], F32)
    inp("b2", [H, 1], F32)
    inp("b3", [H, 1], F32)
    inp("br1", [H, 1], F32)
    inp("br2", [1, 1], F32)
    inp("cb", [H, 1], F32)            # -centers/width, tiled 4x
    inp("recb", [H, 1], F32)          # 1/max(cnt,1) replicated
    inp("gt0", [1, 1], F32)           # 1.0 if cnt > 0 else 0.0

    score_ap = nc.dram_tensor("score", [1, 1], F32, kind="ExternalOutput").ap()

    with tile.TileContext(nc) as tc:
        with ExitStack() as ctx:
            const = ctx.enter_context(tc.tile_pool(name="const", bufs=1))
            sbuf = ctx.enter_context(tc.tile_pool(name="sbuf", bufs=4))
            abuf = ctx.enter_context(tc.tile_pool(name="abuf", bufs=2))
            psZ1 = ctx.enter_context(tc.tile_pool(name="psZ1", bufs=3, space="PSUM"))
            psZ2 = ctx.enter_context(tc.tile_pool(name="psZ2", bufs=3, space="PSUM"))
            psZ3 = ctx.enter_context(tc.tile_pool(name="psZ3", bufs=2, space="PSUM"))

            t = {}
            loads = [
                ("cb", [H, 1], F32), ("onesr", [1, 512], F32R),
                ("hpT", [H, P], F32R), ("W1a", [H, H], F32R),
                ("W1csx", [H, 4 * H], F32R), ("id128", [H, H], F32R),
                ("hlT", [H, L], F32R), ("W1b", [H, H], F32R),
                ("W2", [H, H], F32R), ("W3", [H, H], F32R),
                ("negb3x", [H, 4 * H], F32R),
                ("b1", [H, 1], F32), ("b2", [H, 1], F32), ("b3", [H, 1], F32),
                ("Wr1", [H, H], F32), ("Wr2", [H, 1], F32),
                ("br1", [H, 1], F32), ("br2", [1, 1], F32),
                ("recb", [H, 1], F32), ("gt0", [1, 1], F32),
            ]
            for name, shape, dt in loads[:2]:
                t[name] = const.tile(shape, dt, tag=name, name=name)
                nc.sync.dma_start(out=t[name], in_=d[name])
            # GpSimd cold-start is ~25us; get it going before it gates the
            # first radial-basis group
            warm = const.tile([1, 64], F32, tag="warm", name="warm")
            nc.gpsimd.memset(warm[:, :], 0.0)
            nc.gpsimd.tensor_scalar(out=warm[:, :], in0=warm[:, :], scalar1=1.0,
                                    scalar2=None, op0=ALU.add)
            # persistent notmask tiles: ones-fill once; per-group DMA rewrites rows 32s
            nm4_t = []
            for i_ in range(3):
                nmt = const.tile([H, 512], F32R, tag=f"nm4_{i_}", name=f"nm4_{i_}")
                nc.sync.dma_start(out=nmt[:, :],
                                  in_=d["onesr"].to_broadcast([H, 512]))
                nm4_t.append(nmt)
            rb4s, nm4s, z1s, a1s, a2s, z3s = {}, {}, {}, {}, {}, {}

            def preamble(g):
                deng = nc.sync
                db = sbuf.tile([H, 512], F32, tag="db", name=f"db{g}")
                deng.dma_start(out=db[:, :], in_=d["dbpre"][:, 512 * g:512 * (g + 1)])
                nm4 = sbuf.tile([H, 512], F32R, tag="nm4", name=f"nm4{g}")
                deng.dma_start(out=nm4[:, :], in_=d["nmpre"][:, 512 * g:512 * (g + 1)])
                u1 = sbuf.tile([H, 512], F32, tag="u1", name=f"u1{g}")
                nc.gpsimd.tensor_scalar(out=u1[:, :], in0=db[:, :],
                                        scalar1=1.0 / WIDTH, scalar2=t["cb"][:, :],
                                        op0=ALU.mult, op1=ALU.add)
                u2 = sbuf.tile([H, 512], F32, tag="u2", name=f"u2{g}")
                nc.gpsimd.tensor_tensor(out=u2[:, :], in0=u1[:, :], in1=u1[:, :],
                                        op=ALU.mult)
                rb4 = sbuf.tile([H, 512], F32R, tag="rb4", name=f"rb4{g}")
                nc.scalar.activation(out=rb4[:, :], in_=u2[:, :], func=AF.Exp,
                                     bias=0.0, scale=-0.5)
                rb4s[g], nm4s[g] = rb4, nm4

            def relu_psum_to_sbuf(out_ap, in_ap, bias_ap, use_act, accum=None):
                if use_act:
                    nc.scalar.activation(out=out_ap, in_=in_ap, func=AF.Relu,
                                         bias=bias_ap, scale=1.0,
                                         accum_out=accum)
                else:
                    nc.vector.tensor_scalar(out=out_ap, in0=in_ap,
                                            scalar1=bias_ap, scalar2=0.0,
                                            op0=ALU.add, op1=ALU.max,
                                            accum_out=accum)

            preamble(0)
            preamble(1)
            preamble(2)
            for name, shape, dt in loads[2:]:
                t[name] = const.tile(shape, dt, tag=name, name=name)
                nc.sync.dma_start(out=t[name], in_=d[name])

            # setup: z1_base = W1a.T @ hpT ; hlWb = W1b.T @ hlT + b1
            zb_ps = psZ1.tile([H, P], F32, tag="z1", name="zb_ps")
            nc.tensor.matmul(out=zb_ps[:, :], lhsT=t["W1a"][:, :],
                             rhs=t["hpT"][:, :], start=True, stop=True)
            z1_base = const.tile([H, P], F32R, tag="z1_base", name="z1_base")
            nc.scalar.copy(z1_base[:, :], zb_ps[:, :])

            hl_ps = psZ2.tile([H, L], F32, tag="z2", name="hl_ps")
            nc.tensor.matmul(out=hl_ps[:, :], lhsT=t["W1b"][:, :],
                             rhs=t["hlT"][:, :], start=True, stop=True)
            hlWb = const.tile([H, L], F32, tag="hlWb", name="hlWb")
            nc.scalar.activation(out=hlWb[:, :], in_=hl_ps[:, :],
                                 func=AF.Identity, bias=t["b1"][:, :], scale=1.0)

            acc3a = const.tile([H, TILES // 2], F32, tag="acc3a", name="acc3a")
            acc3b = const.tile([H, TILES // 2], F32, tag="acc3b", name="acc3b")


            z2ps, z3ps, a2ps = {}, {}, {}
            for step in range(TILES + 6):
                # S0: z1 matmuls for tile t0
                t0 = step
                if t0 < TILES:
                    g, s = divmod(t0, 4)
                    if s == 2 and g + 3 < GROUPS:
                        preamble(g + 3)
                    z1 = psZ1.tile([H, 512], F32, tag="z1", name=f"z1_{t0}")
                    z1s[t0] = z1
                    nc.tensor.matmul(out=z1[:, :],
                                     lhsT=t["W1csx"][:, H * s:H * s + H],
                                     rhs=rb4s[g][:, :],
                                     start=True, stop=False)
                    nc.tensor.matmul(out=z1[:, :], lhsT=t["id128"][:, :],
                                     rhs=z1_base[:, :], start=False, stop=True)
                # S1: relu1 for t0-1 (ACT on even tiles, DVE on odd)
                t1 = step - 1
                if 0 <= t1 < TILES:
                    a1 = abuf.tile([H, 512], F32R, tag="a1", name=f"a1_{t1}",
                                   bufs=3)
                    a1s[t1] = a1
                    relu_psum_to_sbuf(a1[:, :], z1s.pop(t1)[:, :],
                                      hlWb[:, t1:t1 + 1], use_act=(t1 % 2 == 0))
                # S2: L2 singles; S3: relu2 singles
                t2 = step - 2
                if 0 <= t2 < TILES:
                    z2 = psZ2.tile([H, 512], F32, tag="z2", name=f"z2_{t2}")
                    nc.tensor.matmul(out=z2[:, :], lhsT=t["W2"][:, :],
                                     rhs=a1s.pop(t2)[:, :], start=True, stop=True)
                    z2ps[t2] = z2
                t3 = step - 3
                if 0 <= t3 < TILES:
                    a2 = abuf.tile([H, 512], F32R, tag="a2", name=f"a2_{t3}")
                    relu_psum_to_sbuf(a2[:, :], z2ps.pop(t3)[:, :],
                                      t["b2"][:, :], use_act=(t3 % 8 in (1, 3, 5)))
                    a2ps[t3] = a2
                # S4: L3 + inject into z3-pair halves; relu3+accum per pair
                t4 = step - 4
                if 0 <= t4 < TILES:
                    g4, s4 = divmod(t4, 4)
                    z3 = psZ3.tile([H, 512], F32, tag="z3", name=f"z3_{t4}")
                    z3ps[t4] = z3
                    nc.tensor.matmul(out=z3[:, :], lhsT=t["W3"][:, :],
                                     rhs=a2ps.pop(t4)[:, :],
                                     start=True, stop=False)
                    nc.tensor.matmul(out=z3[:, :],
                                     lhsT=t["negb3x"][:, H * s4:H * s4 + H],
                                     rhs=nm4s[g4][:, :],
                                     start=False, stop=True)
                t5 = step - 5
                if 0 <= t5 < TILES:
                    a3 = abuf.tile([H, 512], F32, tag="a3", name=f"a3_{t5}")
                    use_act = (t5 % 2 == 0)
                    accum = (acc3a if use_act else acc3b)[:, t5 // 2:t5 // 2 + 1]
                    z3ap = z3ps.pop(t5)
                    if use_act:
                        nc.scalar.activation(out=a3[:, :], in_=z3ap[:, :],
                                             func=AF.Relu, bias=0.0, scale=1.0,
                                             accum_out=accum)
                    else:
                        nc.vector.tensor_scalar(out=a3[:, :], in0=z3ap[:, :],
                                                scalar1=0.0, scalar2=0.0,
                                                op0=ALU.max, op1=ALU.add,
                                                accum_out=accum)

            # ---- head ----
            tota = const.tile([H, 1], F32, tag="tota", name="tota")
            totb = const.tile([H, 1], F32, tag="totb", name="totb")
            nc.vector.tensor_reduce(out=tota[:, :], in_=acc3a[:, :],
                                    axis=mybir.AxisListType.X, op=ALU.add)
            nc.vector.tensor_reduce(out=totb[:, :], in_=acc3b[:, :],
                                    axis=mybir.AxisListType.X, op=ALU.add)
            tot = const.tile([H, 1], F32, tag="tot", name="tot")
            nc.vector.tensor_tensor(out=tot[:, :], in0=tota[:, :],
                                    in1=totb[:, :], op=ALU.add)
            repr_ = const.tile([H, 1], F32, tag="repr", name="repr_")
            nc.vector.tensor_tensor(out=repr_[:, :], in0=tot[:, :],
                                    in1=t["recb"][:, :], op=ALU.mult)
            r1_ps = psZ2.tile([H, 1], F32, tag="z2", name="r1_ps")
            nc.tensor.matmul(out=r1_ps[:, :], lhsT=t["Wr1"][:, :],
                             rhs=repr_[:, :], start=True, stop=True)
            r1 = const.tile([H, 1], F32, tag="r1", name="r1")
            nc.scalar.activation(out=r1[:, :], in_=r1_ps[:, :], func=AF.Relu,
                                 bias=t["br1"][:, :], scale=1.0)
            sc_ps = psZ3.tile([1, 1], F32, tag="z3", name="sc_ps")
            nc.tensor.matmul(out=sc_ps[:, :], lhsT=t["Wr2"][:, :],
                             rhs=r1[:, :], start=True, stop=True)
            sc = const.tile([1, 1], F32, tag="sc", name="sc")
            nc.scalar.activation(out=sc[:, :], in_=sc_ps[:, :], func=AF.Identity,
                                 bias=t["br2"][:, :], scale=1.0)
            scf = const.tile([1, 1], F32, tag="scf", name="scf")
            nc.vector.tensor_tensor(out=scf[:, :], in0=sc[:, :],
                                    in1=t["gt0"][:, :], op=ALU.mult)
            nc.sync.dma_start(out=score_ap, in_=scf[:, :])

    nc.compile()
    return nc


def _get_nc():
    if "nc" not in _CACHE:
        _CACHE["nc"] = _build_nc()
    return _CACHE["nc"]


def kernel(protein_pos, ligand_pos, prot_emb, lig_emb,
           W1, b1, W2, b2, W3, b3, Wr1, br1, Wr2, br2,
           protein_atom_type, ligand_atom_type, protein_batch, ligand_batch):
    protein_pos = np.asarray(protein_pos, dtype=np.float32).reshape(B, P, 3)
    ligand_pos = np.asarray(ligand_pos, dtype=np.float32).reshape(B, L, 3)
    prot_emb = np.asarray(prot_emb, dtype=np.float32)
    lig_emb = np.asarray(lig_emb, dtype=np.float32)
    W1 = np.asarray(W1, dtype=np.float32)
    ptype = np.asarray(protein_atom_type).reshape(B, P)
    ltype = np.asarray(ligand_atom_type).reshape(B, L)

    W1a = np.ascontiguousarray(W1[0:H, :])
    W1b = np.ascontiguousarray(W1[H:2 * H, :])
    W1c = np.ascontiguousarray(W1[2 * H:2 * H + RB, :])
    W1csx = np.zeros((H, 4 * H), dtype=np.float32)
    negb3x = np.zeros((H, 4 * H), dtype=np.float32)
    for s in range(4):
        W1csx[32 * s:32 * s + 32, H * s:H * (s + 1)] = W1c
        negb3x[32 * s, H * s:H * (s + 1)] = -1e9
        negb3x[32 * s + 1, H * s:H * (s + 1)] = np.asarray(b3, np.float32).reshape(H)
    centers = np.linspace(0.0, CUTOFF, RB, dtype=np.float32)
    cb = np.tile(-centers / np.float32(WIDTH), 4).reshape(H, 1).astype(np.float32)

    common = {
        "W1a": W1a, "W1b": W1b, "W1csx": W1csx,
        "W2": np.asarray(W2, np.float32), "W3": np.asarray(W3, np.float32),
        "Wr1": np.asarray(Wr1, np.float32),
        "Wr2": np.asarray(Wr2, np.float32).reshape(H, 1),
        "negb3x": negb3x, "id128": np.eye(H, dtype=np.float32),
        "onesr": np.ones((1, 512), np.float32),
        "b1": np.asarray(b1, np.float32).reshape(H, 1),
        "b2": np.asarray(b2, np.float32).reshape(H, 1),
        "b3": np.asarray(b3, np.float32).reshape(H, 1),
        "br1": np.asarray(br1, np.float32).reshape(H, 1),
        "br2": np.asarray(br2, np.float32).reshape(1, 1),
        "cb": cb,
    }

    in_maps = []
    for b in range(B):
        hpT = np.ascontiguousarray(prot_emb[ptype[b]].T)
        hlT = np.ascontiguousarray(lig_emb[ltype[b]].T)
        diff = protein_pos[b][:, None, :] - ligand_pos[b][None, :, :]
        dist = np.sqrt((diff * diff).sum(-1, dtype=np.float32))
        distT = dist.T.reshape(GROUPS, 4, P)          # [g, s, p]
        nm = (distT >= np.float32(CUTOFF)).astype(np.float32)
        # pre-broadcast dist: rows 32s..32s+31 of group g = distT[g, s]
        dbpre = np.repeat(distT, 32, axis=1).transpose(1, 0, 2).reshape(H, GROUPS * P)
        nmpre = np.zeros((H, GROUPS, P), dtype=np.float32)
        for s in range(4):
            nmpre[32 * s] = nm[:, s, :]
            nmpre[32 * s + 1] = 1.0
        nmpre = nmpre.reshape(H, GROUPS * P)
        cnt = float(NPAIR - nm.sum())
        recb = np.full((H, 1), 1.0 / max(cnt, 1.0), dtype=np.float32)
        gt0 = np.full((1, 1), 1.0 if cnt > 0 else 0.0, dtype=np.float32)
        m = dict(common)
        m.update({"hpT": hpT, "hlT": hlT,
                  "dbpre": np.ascontiguousarray(dbpre),
                  "nmpre": np.ascontiguousarray(nmpre),
                  "recb": recb, "gt0": gt0})
        in_maps.append(m)

    nc = _get_nc()
    res = bass_utils.run_bass_kernel_spmd(nc, in_maps,
                                          core_ids=list(range(N_CORES)))
    out = np.array([res.results[b]["score"][0, 0] for b in range(B)],
                   dtype=np.float32)
    return out



# revision 7
# speedup vs baseline: 1.6785x; 1.6785x over previous
"""DockingScorePredictor Trainium2 kernel — compacted-pair MLP.

Data-parallel over complexes: 8 cores, one complex (512 protein x 64 ligand
atoms) per core.  The host drops masked pairs (dist >= cutoff, ~55%) and
packs the survivors into tiles of 512 pairs.  Because atom-type vocabularies
are tiny (20/16), the whole first layer collapses into ONE K=68 matmul per
tile: rhs rows = [32 radial-basis | 20 protein-type one-hot | 16 ligand-type
one-hot], lhsT rows = [W1c | prot_emb@W1a | lig_emb@W1b]; b1 rides the relu
bias port.  Per tile the MLP is then 3 matmuls (z1, W2, W3) instead of the
5 of the uncompacted formulation, and only ~31 tiles run instead of 64.

Each relu is one ACT/DVE op over a single-bank [128,512] PSUM tile (engine
reads cannot cross PSUM bank boundaries).  relu1 -> DVE, relu2 -> ACT,
relu3+pair-sum alternates engines by tile parity via the accum_out port.
Padding columns (all-zero rhs) produce a constant h3 = relu(W3^T relu(W2^T
relu(b1) + b2) + b3) which the host pre-computes (with matching bf16
quantization) and subtracts from the pair-sum.  All MLP matmuls are bf16
(FWL keeps LDWEIGHTS off the critical path); rel tolerance is 2e-2 and
bf16 quantization lands ~2e-3.
"""
import numpy as np
import ml_dtypes
from contextlib import ExitStack

import concourse.bass as bass
import concourse.bacc as bacc
import concourse.tile as tile
from concourse import mybir
from concourse import bass_utils

F32 = mybir.dt.float32
BF16 = mybir.dt.bfloat16
AF = mybir.ActivationFunctionType
ALU = mybir.AluOpType

B, P, L = 8, 512, 64
H, RB = 128, 32
NPT, NLT = 20, 16
CUTOFF = 8.0
N_CORES = 8
WIDTH = 0.5 * CUTOFF / RB + 1e-8
K1 = RB + NPT + NLT            # 68 rows of the fused layer-1 rhs
TCOLS = 512                    # pair columns per tile (one PSUM bank)

_CACHE = {}
BF = ml_dtypes.bfloat16


def _build_nc(nt):
    nc = bacc.Bacc("TRN2", target_bir_lowering=False, debug=False,
                   num_devices=N_CORES)
    d = {}

    def inp(name, shape, dt):
        d[name] = nc.dram_tensor(name, shape, dt, kind="ExternalInput").ap()

    inp("rhs_all", [K1, nt * TCOLS], BF16)
    inp("lhs1", [K1, H], BF16)
    inp("W2", [H, H], BF16)
    inp("W3", [H, H], BF16)
    inp("Wr1", [H, H], F32)
    inp("Wr2", [H, 1], F32)
    inp("b1", [H, 1], F32)
    inp("b2", [H, 1], F32)
    inp("b3", [H, 1], F32)
    inp("b3row", [1, H], BF16)
    inp("onesr", [1, TCOLS], BF16)
    inp("br1", [H, 1], F32)
    inp("br2", [1, 1], F32)
    inp("padc3", [H, 1], F32)     # n_pad * c3
    inp("recb", [H, 1], F32)      # 1/max(cnt,1)
    inp("gt0", [1, 1], F32)       # 1.0 if cnt > 0 else 0.0

    score_ap = nc.dram_tensor("score", [1, 1], F32, kind="ExternalOutput").ap()

    nacc = (nt + 1) // 2

    with tile.TileContext(nc) as tc:
        with ExitStack() as ctx:
            const = ctx.enter_context(tc.tile_pool(name="const", bufs=1))
            rhsp = ctx.enter_context(tc.tile_pool(name="rhsp", bufs=3))
            abuf = ctx.enter_context(tc.tile_pool(name="abuf", bufs=2))
            psZ1 = ctx.enter_context(tc.tile_pool(name="psZ1", bufs=3, space="PSUM"))
            psZ2 = ctx.enter_context(tc.tile_pool(name="psZ2", bufs=3, space="PSUM"))
            psZ3 = ctx.enter_context(tc.tile_pool(name="psZ3", bufs=2, space="PSUM"))

            t = {}
            loads = [
                ("lhs1", [K1, H], BF16), ("W2", [H, H], BF16),
                ("W3", [H, H], BF16),
                ("b1", [H, 1], F32), ("b2", [H, 1], F32), ("b3", [H, 1], F32),
                ("b3row", [1, H], BF16), ("onesr", [1, TCOLS], BF16),
                ("Wr1", [H, H], F32), ("Wr2", [H, 1], F32),
                ("br1", [H, 1], F32), ("br2", [1, 1], F32),
                ("padc3", [H, 1], F32), ("recb", [H, 1], F32),
                ("gt0", [1, 1], F32),
            ]
            for name, shape, dt in loads:
                t[name] = const.tile(shape, dt, tag=name, name=name)
                nc.sync.dma_start(out=t[name], in_=d[name])

            # prime the ACT function table (Relu) so the ~1.5us table load
            # overlaps the initial DMAs instead of stalling relu2(0)
            warm = const.tile([1, 1], F32, tag="warm", name="warm")
            nc.vector.memset(warm[:, :], 0.0)
            nc.scalar.activation(out=warm[:, :], in_=warm[:, :], func=AF.Relu,
                                 bias=0.0, scale=1.0)

            accA = const.tile([H, nacc], F32, tag="accA", name="accA")
            accD = const.tile([H, nacc], F32, tag="accD", name="accD")

            rhs_t, z1_t, a1_t, z2_t, a2_t, z3_t = {}, {}, {}, {}, {}, {}

            for step in range(nt + 8):
                # S6: relu3 + pair-sum accumulate (alternating engine).
                # DVE's accumulator needs op1=add, so its tiles get b3
                # pre-injected into PSUM by the K=1 matmul below.
                ti = step - 7
                if 0 <= ti < nt:
                    a3 = abuf.tile([H, TCOLS], BF16, tag="a3", name=f"a3_{ti}")
                    z3 = z3_t.pop(ti)
                    acc = (accA if ti % 2 == 0 else accD)[:, ti // 2:ti // 2 + 1]
                    if ti % 2 == 0:
                        nc.scalar.activation(out=a3[:, :], in_=z3[:, :],
                                             func=AF.Relu, bias=t["b3"][:, :],
                                             scale=1.0, accum_out=acc)
                    else:
                        nc.vector.tensor_scalar(out=a3[:, :], in0=z3[:, :],
                                                scalar1=0.0, scalar2=0.0,
                                                op0=ALU.max, op1=ALU.add,
                                                accum_out=acc)
                # S5: layer-3 matmul (+ b3 row-inject on DVE-relu tiles)
                ti = step - 6
                if 0 <= ti < nt:
                    z3 = psZ3.tile([H, TCOLS], F32, tag="z3", name=f"z3_{ti}")
                    last = ti % 2 == 0
                    nc.tensor.matmul(out=z3[:, :], lhsT=t["W3"][:, :],
                                     rhs=a2_t.pop(ti)[:, :],
                                     start=True, stop=last)
                    if not last:
                        nc.tensor.matmul(out=z3[:, :], lhsT=t["b3row"][:, :],
                                         rhs=t["onesr"][:, :],
                                         start=False, stop=True)
                    z3_t[ti] = z3
                # S4: relu2 (ACT)
                ti = step - 5
                if 0 <= ti < nt:
                    a2 = abuf.tile([H, TCOLS], BF16, tag="a2", name=f"a2_{ti}")
                    nc.scalar.activation(out=a2[:, :], in_=z2_t.pop(ti)[:, :],
                                         func=AF.Relu, bias=t["b2"][:, :],
                                         scale=1.0)
                    a2_t[ti] = a2
                # S3: layer-2 matmul
                ti = step - 4
                if 0 <= ti < nt:
                    z2 = psZ2.tile([H, TCOLS], F32, tag="z2", name=f"z2_{ti}")
                    nc.tensor.matmul(out=z2[:, :], lhsT=t["W2"][:, :],
                                     rhs=a1_t.pop(ti)[:, :],
                                     start=True, stop=True)
                    z2_t[ti] = z2
                # S2: relu1 (DVE), bias b1 via scalar port
                ti = step - 3
                if 0 <= ti < nt:
                    a1 = abuf.tile([H, TCOLS], BF16, tag="a1", name=f"a1_{ti}")
                    nc.vector.tensor_scalar(out=a1[:, :], in0=z1_t.pop(ti)[:, :],
                                            scalar1=t["b1"][:, :], scalar2=0.0,
                                            op0=ALU.add, op1=ALU.max)
                    a1_t[ti] = a1
                # S1: fused layer-1 matmul (rb + type one-hots)
                ti = step - 2
                if 0 <= ti < nt:
                    z1 = psZ1.tile([H, TCOLS], F32, tag="z1", name=f"z1_{ti}")
                    rhs = rhs_t[ti // 2]
                    nc.tensor.matmul(out=z1[:, :], lhsT=t["lhs1"][:, :],
                                     rhs=rhs[:, TCOLS * (ti % 2):TCOLS * (ti % 2 + 1)],
                                     start=True, stop=True)
                    if ti % 2 == 1 or ti == nt - 1:
                        del rhs_t[ti // 2]
                    z1_t[ti] = z1
                # S0: rhs DMA, two tiles per transfer
                ti = step
                if 0 <= ti < nt and ti % 2 == 0:
                    g = ti // 2
                    w = min(2 * TCOLS, (nt - ti) * TCOLS)
                    rhs = rhsp.tile([K1, 2 * TCOLS], BF16, tag="rhs",
                                    name=f"rhs{g}")
                    nc.sync.dma_start(
                        out=rhs[:, 0:w],
                        in_=d["rhs_all"][:, 2 * TCOLS * g:2 * TCOLS * g + w])
                    rhs_t[g] = rhs

            # ---- head ----
            totA = const.tile([H, 1], F32, tag="totA", name="totA")
            totD = const.tile([H, 1], F32, tag="totD", name="totD")
            nc.vector.tensor_reduce(out=totA[:, :], in_=accA[:, :],
                                    axis=mybir.AxisListType.X, op=ALU.add)
            nc.vector.tensor_reduce(out=totD[:, :], in_=accD[:, :],
                                    axis=mybir.AxisListType.X, op=ALU.add)
            tot = const.tile([H, 1], F32, tag="tot", name="tot")
            nc.vector.tensor_tensor(out=tot[:, :], in0=totA[:, :],
                                    in1=totD[:, :], op=ALU.add)
            # repr = (tot - padc3) * recb
            repr_ = const.tile([H, 1], F32, tag="repr", name="repr_")
            nc.vector.scalar_tensor_tensor(out=repr_[:, :], in0=tot[:, :],
                                           scalar=t["padc3"][:, :],
                                           in1=t["recb"][:, :],
                                           op0=ALU.subtract, op1=ALU.mult)
            r1_ps = psZ2.tile([H, 1], F32, tag="z2", name="r1_ps")
            nc.tensor.matmul(out=r1_ps[:, :], lhsT=t["Wr1"][:, :],
                             rhs=repr_[:, :], start=True, stop=True)
            r1 = const.tile([H, 1], F32, tag="r1", name="r1")
            nc.scalar.activation(out=r1[:, :], in_=r1_ps[:, :], func=AF.Relu,
                                 bias=t["br1"][:, :], scale=1.0)
            sc_ps = psZ3.tile([1, 1], F32, tag="z3", name="sc_ps")
            nc.tensor.matmul(out=sc_ps[:, :], lhsT=t["Wr2"][:, :],
                             rhs=r1[:, :], start=True, stop=True)
            scf = const.tile([1, 1], F32, tag="scf", name="scf")
            nc.vector.scalar_tensor_tensor(out=scf[:, :], in0=sc_ps[:, :],
                                           scalar=t["br2"][:, :],
                                           in1=t["gt0"][:, :],
                                           op0=ALU.add, op1=ALU.mult)
            nc.sync.dma_start(out=score_ap, in_=scf[:, :])

    nc.compile()
    return nc


def _get_nc(nt):
    key = ("nc", nt)
    if key not in _CACHE:
        _CACHE[key] = _build_nc(nt)
    return _CACHE[key]


def kernel(protein_pos, ligand_pos, prot_emb, lig_emb,
           W1, b1, W2, b2, W3, b3, Wr1, br1, Wr2, br2,
           protein_atom_type, ligand_atom_type, protein_batch, ligand_batch):
    protein_pos = np.asarray(protein_pos, dtype=np.float32).reshape(B, P, 3)
    ligand_pos = np.asarray(ligand_pos, dtype=np.float32).reshape(B, L, 3)
    prot_emb = np.asarray(prot_emb, dtype=np.float32)
    lig_emb = np.asarray(lig_emb, dtype=np.float32)
    W1 = np.asarray(W1, dtype=np.float32)
    W2 = np.asarray(W2, dtype=np.float32)
    W3 = np.asarray(W3, dtype=np.float32)
    b1 = np.asarray(b1, dtype=np.float32).reshape(H)
    b2 = np.asarray(b2, dtype=np.float32).reshape(H)
    b3 = np.asarray(b3, dtype=np.float32).reshape(H)
    ptype = np.asarray(protein_atom_type).astype(np.int64).reshape(B, P)
    ltype = np.asarray(ligand_atom_type).astype(np.int64).reshape(B, L)

    # fused layer-1 weights: [W1c | prot_emb@W1a | lig_emb@W1b]
    W1a, W1b, W1c = W1[0:H], W1[H:2 * H], W1[2 * H:2 * H + RB]
    lhs1 = np.zeros((K1, H), dtype=np.float32)
    lhs1[0:RB] = W1c
    lhs1[RB:RB + NPT] = prot_emb @ W1a
    lhs1[RB + NPT:K1] = lig_emb @ W1b
    lhs1_bf = lhs1.astype(BF)
    W2_bf = W2.astype(BF)
    W3_bf = W3.astype(BF)

    # padding column output, with device-matching bf16 quantization
    a1p = np.maximum(b1, 0.0).astype(BF).astype(np.float32)
    a2p = np.maximum(W2_bf.astype(np.float32).T @ a1p + b2, 0.0)
    a2p = a2p.astype(BF).astype(np.float32)
    c3 = np.maximum(W3_bf.astype(np.float32).T @ a2p + b3, 0.0)

    centers = np.linspace(0.0, CUTOFF, RB, dtype=np.float32)

    # per-complex compaction
    rb_l, pt_l, lt_l, nv_l = [], [], [], []
    for b in range(B):
        diff = protein_pos[b][:, None, :] - ligand_pos[b][None, :, :]
        dist = np.sqrt((diff * diff).sum(-1, dtype=np.float32))
        pidx, lidx = np.nonzero(dist < np.float32(CUTOFF))
        dv = dist[pidx, lidx]
        rb_l.append(np.exp(-0.5 * ((dv[:, None] - centers) / WIDTH) ** 2,
                           dtype=np.float32))
        pt_l.append(ptype[b][pidx])
        lt_l.append(ltype[b][lidx])
        nv_l.append(len(dv))

    nt = max(2, -(-max(nv_l) // TCOLS))
    cols = nt * TCOLS

    common = {
        "lhs1": lhs1_bf, "W2": W2_bf, "W3": W3_bf,
        "Wr1": np.asarray(Wr1, np.float32),
        "Wr2": np.asarray(Wr2, np.float32).reshape(H, 1),
        "b1": b1.reshape(H, 1), "b2": b2.reshape(H, 1), "b3": b3.reshape(H, 1),
        "b3row": b3.reshape(1, H).astype(BF),
        "onesr": np.ones((1, TCOLS), dtype=BF),
        "br1": np.asarray(br1, np.float32).reshape(H, 1),
        "br2": np.asarray(br2, np.float32).reshape(1, 1),
    }

    in_maps = []
    for b in range(B):
        nv = nv_l[b]
        rhs = np.zeros((K1, cols), dtype=BF)
        rhs[0:RB, :nv] = rb_l[b].T.astype(BF)
        ar = np.arange(nv)
        onehot = np.zeros((NPT + NLT, nv), dtype=BF)
        onehot[pt_l[b], ar] = 1.0
        onehot[NPT + lt_l[b], ar] = 1.0
        rhs[RB:K1, :nv] = onehot
        m = dict(common)
        m.update({
            "rhs_all": rhs,
            "padc3": ((cols - nv) * c3).reshape(H, 1).astype(np.float32),
            "recb": np.full((H, 1), 1.0 / max(nv, 1.0), dtype=np.float32),
            "gt0": np.full((1, 1), 1.0 if nv > 0 else 0.0, dtype=np.float32),
        })
        in_maps.append(m)

    nc = _get_nc(nt)
    res = bass_utils.run_bass_kernel_spmd(nc, in_maps,
                                          core_ids=list(range(N_CORES)))
    out = np.array([res.results[b]["score"][0, 0] for b in range(B)],
                   dtype=np.float32)
    return out
